# revision 43
# baseline (speedup 1.0000x reference)
"""HGRN2 attention forward on 8 Trainium2 NeuronCores.

Sharding: phase 1 is head-parallel (16 (batch, head) pairs -> 2 per core),
phase 2 is token-parallel (8192 token rows -> 1024 per core). The host
reshuffles the per-head scan outputs between the two SPMD launches.

The sequential gated scan is evaluated chunk-parallel (chunk C=64) with a
factored ("pre-scale") state P where the true state is S_u = egcC_u * P_u:
  P_{u+1} = egcC_u * P_u + kt_{u+1}^T v_{u+1}      (one STT op per head)
  o_u     = scale * (tril(qt^T kt) @ v + (qt * egcC_{u-1})^T @ P_{u-1})
with egc the per-chunk inclusive cumprod of sigmoid(z_f), qt = silu(z_q)*egc,
kt = (1-sigmoid(z_f))/egc. All matmuls run on the TensorEngine in bf16
(fp32 PSUM accumulation); v is produced token-major directly by using the
x tile as the stationary operand; the two heads share PSUM tiles for the
masked-attention, k-transpose and output stages so DVE/ACT ops and DMAs
are issued once per chunk instead of once per (chunk, head).
"""

import numpy as np
import ml_dtypes
from contextlib import ExitStack

import concourse.bass as bass
import concourse.mybir as mybir
import concourse.tile as tile
from concourse import bacc
from concourse.bass_utils import run_bass_kernel_spmd

F32 = mybir.dt.float32
BF16 = mybir.dt.bfloat16
AF = mybir.ActivationFunctionType
OP = mybir.AluOpType
PSUM = bass.MemorySpace.PSUM

B, T, D = 2, 4096, 1024
H, DF, DI = 8, 128, 128
EPS = 1e-5
SCALE = float(DF) ** -0.5
NCORES = 8
NH = 2              # heads per core
C = 64              # scan chunk length
TT = 512            # phase-1 token tile
NKT = D // 128      # contraction tiles
NTT = T // TT       # token tiles per core (phase 1)
NCHUNK = TT // C    # chunks per token tile
NBLK = TT // 128    # 128-token blocks per tile (token-major v)
ROWS2 = (B * T) // NCORES  # phase-2 token rows per core

NPBF16 = ml_dtypes.bfloat16


def _mk_nc():
    return bacc.Bacc(
        "TRN2",
        target_bir_lowering=False,
        debug=False,
        num_devices=NCORES,
    )


def _build_phase1(ntt=NTT, nh=NH):
    nc = _mk_nc()
    t_len = ntt * TT
    xT = nc.dram_tensor("xT", [D, t_len], BF16, kind="ExternalInput")
    wqT = nc.dram_tensor("wqT", [D, nh * DF], BF16, kind="ExternalInput")
    wfT = nc.dram_tensor("wfT", [D, nh * DF], BF16, kind="ExternalInput")
    wiT = nc.dram_tensor("wiT", [D, nh * DI], BF16, kind="ExternalInput")
    ident = nc.dram_tensor("ident", [128, 128], BF16, kind="ExternalInput")
    maskT = nc.dram_tensor("maskT", [C, nh * C], F32, kind="ExternalInput")
    seg = nc.dram_tensor("seg", [128, TT], F32, kind="ExternalInput")
    o_out = nc.dram_tensor("o", [t_len, nh * DI], BF16, kind="ExternalOutput")

    with ExitStack() as ctx:
        tc = ctx.enter_context(tile.TileContext(nc))
        const = ctx.enter_context(tc.tile_pool(name="const", bufs=1))
        wpool = ctx.enter_context(tc.tile_pool(name="w", bufs=1))
        xpool = ctx.enter_context(tc.tile_pool(name="x", bufs=2))
        work = ctx.enter_context(tc.tile_pool(name="work", bufs=3))
        small = ctx.enter_context(tc.tile_pool(name="small", bufs=6))
        spool = ctx.enter_context(tc.tile_pool(name="state", bufs=2))
        ps_proj = ctx.enter_context(tc.tile_pool(name="ps_proj", bufs=2, space=PSUM))
        # v (projection epoch) and o (chunk epoch) alternate through one
        # shared 2-buf pool -> each is effectively double-buffered
        ps_vo = ctx.enter_context(tc.tile_pool(name="ps_vo", bufs=2, space=PSUM))
        ps_at = ctx.enter_context(tc.tile_pool(name="ps_at", bufs=1, space=PSUM))
        ps_tr = ctx.enter_context(tc.tile_pool(name="ps_tr", bufs=1, space=PSUM))
        ps_s = ctx.enter_context(tc.tile_pool(name="ps_s", bufs=2, space=PSUM))

        id_sb = const.tile([128, 128], BF16, tag="id")
        nc.sync.dma_start(id_sb[:], ident[:])
        mT_sb = const.tile([C, nh * C], F32, tag="mT")
        nc.sync.dma_start(mT_sb[:], maskT[:])
        seg_sb = const.tile([128, TT], F32, tag="seg")
        nc.sync.dma_start(seg_sb[:], seg[:])

        w_sb = {}
        for name, dram in (("q", wqT), ("f", wfT), ("i", wiT)):
            wt = wpool.tile([128, NKT, nh * DF], BF16, tag=f"w{name}")
            nc.sync.dma_start(wt[:], dram[:].rearrange("(k p) m -> p k m", p=128))
            w_sb[name] = wt

        P = []          # factored state per head (true S = egcC * P)
        eglast = []     # egc at last token of previous tile, per head
        for h in range(nh):
            s0 = spool.tile([DF, DI], BF16, tag=f"s{h}")
            nc.vector.memset(s0[:], 0.0)
            P.append(s0)
            el = spool.tile([128, 1], F32, tag=f"el{h}")
            nc.vector.memset(el[:], 1.0)
            eglast.append(el)

        def emit_vblock(xt, vtm, blk):
            vt_ps = ps_vo.tile([128, nh * DI], F32, tag="vo", name="vt_ps")
            for kt in range(NKT):
                nc.tensor.matmul(
                    vt_ps[:],
                    xt[:, kt, blk * 128:(blk + 1) * 128],
                    w_sb["i"][:, kt, :],
                    start=(kt == 0), stop=(kt == NKT - 1),
                )
            nc.scalar.copy(vtm[:, 2 * blk, :], vt_ps[0:C, :])
            nc.scalar.copy(vtm[:, 2 * blk + 1, :], vt_ps[C:128, :])

        def emit_zq(xt, hd, h):
            hs = slice(h * DF, (h + 1) * DF)
            zq = ps_proj.tile([128, TT], F32, tag="proj")
            for kt in range(NKT):
                nc.tensor.matmul(
                    zq[:], w_sb["q"][:, kt, hs], xt[:, kt, :],
                    start=(kt == 0), stop=(kt == NKT - 1),
                )
            # single ACT reader so the PSUM bank frees fast; Silu and Tanh
            # share one table set (silu_and_others) -> no table switches
            q_sb = work.tile([128, TT], F32, tag=f"q{h}")
            nc.scalar.activation(q_sb[:], zq[:], AF.Silu)
            hd[("q", h)] = q_sb

        def emit_zf(xt, hd, h):
            hs = slice(h * DF, (h + 1) * DF)
            zf = ps_proj.tile([128, TT], F32, tag="proj")
            for kt in range(NKT):
                nc.tensor.matmul(
                    zf[:], w_sb["f"][:, kt, hs], xt[:, kt, :],
                    start=(kt == 0), stop=(kt == NKT - 1),
                )
            # sigmoid via tanh (same ACT table as Silu):
            # sig = 0.5*tanh(z/2) + 0.5, k = 1 - sig = -0.5*tanh(z/2) + 0.5
            th = work.tile([128, TT], F32, tag="th")
            nc.scalar.activation(th[:], zf[:], AF.Tanh, scale=0.5)
            sig = work.tile([128, TT], F32, tag="sig")
            nc.vector.tensor_scalar(sig[:], th[:], 0.5, 0.5, OP.mult, OP.add)
            k_sb = work.tile([128, TT], F32, tag="k")
            nc.gpsimd.tensor_scalar(
                k_sb[:], th[:], -0.5, 0.5, OP.mult, OP.add
            )
            # per-chunk inclusive cumprod of sigmoid(zf) via scan with
            # reset at chunk starts (seg==0 there); prep on GpSimd
            d0 = work.tile([128, TT], F32, tag="d0")
            nc.gpsimd.tensor_tensor(d0[:], sig[:], seg_sb[:], OP.mult)
            d1 = work.tile([128, TT], F32, tag="d1")
            nc.gpsimd.tensor_tensor(d1[:], sig[:], d0[:], OP.subtract)
            egc = work.tile([128, TT], F32, tag=f"egc{h}")
            nc.vector.tensor_tensor_scan(
                egc[:], d0[:], d1[:], 0.0, OP.mult, OP.add
            )
            ep = work.tile([128, TT], F32, tag="ep")
            nc.vector.reciprocal_approx_fast(ep[:], egc[:])
            q_sb = hd[("q", h)]
            qt_sb = work.tile([128, TT], BF16, tag=f"qt{h}")
            nc.vector.tensor_tensor(qt_sb[:], q_sb[:], egc[:], OP.mult)
            kt_sb = work.tile([128, TT], BF16, tag=f"kt{h}")
            nc.vector.tensor_tensor(kt_sb[:], k_sb[:], ep[:], OP.mult)
            # qtp_t = qt_t * egcC_{chunk(t)-1}: extra decay from chunk entry,
            # for the o += qtp^T P term
            qtp_sb = work.tile([128, TT], BF16, tag=f"qtp{h}")
            nc.vector.tensor_scalar(
                qtp_sb[:, 0:C], qt_sb[:, 0:C], eglast[h][:], None, OP.mult
            )
            egr = egc[:].rearrange("p (c t) -> p c t", c=NCHUNK)
            egv = egr[:, 0:NCHUNK - 1, C - 1:C].broadcast_to(
                (128, NCHUNK - 1, C)
            )
            nc.vector.tensor_tensor(
                qtp_sb[:].rearrange("p (c t) -> p c t", c=NCHUNK)[:, 1:NCHUNK, :],
                qt_sb[:].rearrange("p (c t) -> p c t", c=NCHUNK)[:, 1:NCHUNK, :],
                egv, OP.mult,
            )
            hd[h] = (qt_sb, kt_sb, qtp_sb, egc, eglast[h])
            # next tile's gates and chunks use this tile's last egc column
            el = spool.tile([128, 1], F32, tag=f"el{h}")
            nc.gpsimd.tensor_copy(el[:], egc[:, TT - 1:TT])
            eglast[h] = el

        def emit_chunk_a(hd, st, u):
            # A^T[s,t] per head, side by side in one PSUM tile
            at_ps = ps_at.tile([C, nh * C], F32, tag="at")
            sl = slice(u * C, (u + 1) * C)
            for h in range(nh):
                nc.tensor.matmul(
                    at_ps[:, h * C:(h + 1) * C],
                    hd[h][1][:, sl], hd[h][0][:, sl],
                    start=True, stop=True,
                )
            atm = small.tile([C, nh * C], BF16, tag="atm")
            nc.vector.tensor_tensor(atm[:], at_ps[:], mT_sb[:], OP.mult)

            # token-major k for both heads via PE transpose
            kht_ps = ps_tr.tile([C, nh * DF], BF16, tag="tr")
            for h in range(nh):
                nc.tensor.transpose(
                    kht_ps[:, h * DF:(h + 1) * DF], hd[h][1][:, sl], id_sb[:]
                )
            kht = small.tile([C, nh * DF], BF16, tag="kht")
            nc.scalar.copy(kht[:], kht_ps[:])
            st[u] = (atm, kht)

        def emit_chunk_b(hd, st, vtm, tt, u):
            atm, kht = st.pop(u)
            sl = slice(u * C, (u + 1) * C)
            # o = A @ v + qtp^T @ P (token-major [C, nh*DI])
            o_ps = ps_vo.tile([C, nh * DI], F32, tag="vo", name="o_ps")
            for h in range(nh):
                od = slice(h * DI, (h + 1) * DI)
                nc.tensor.matmul(
                    o_ps[:, od], atm[:, h * C:(h + 1) * C], vtm[:, u, od],
                    start=True, stop=False,
                )
                nc.tensor.matmul(
                    o_ps[:, od], hd[h][2][:, sl], P[h][:],
                    start=False, stop=True,
                )

            # P' = egcC_{u-1} * P + kt^T @ v
            for h in range(nh):
                od = slice(h * DI, (h + 1) * DI)
                s_ps = ps_s.tile([DF, DI], F32, tag="sps")
                nc.tensor.matmul(
                    s_ps[:], kht[:, h * DF:(h + 1) * DF], vtm[:, u, od],
                    start=True, stop=True,
                )
                egc = hd[h][3]
                dec = hd[h][4][:] if u == 0 else egc[:, u * C - 1:u * C]
                s_new = spool.tile([DF, DI], BF16, tag=f"s{h}")
                nc.vector.scalar_tensor_tensor(
                    s_new[:], P[h][:], dec, s_ps[:], OP.mult, OP.add,
                )
                P[h] = s_new

            o_sb = small.tile([C, nh * DI], BF16, tag="osb")
            nc.scalar.activation(o_sb[:], o_ps[:], AF.Copy, scale=SCALE)
            nc.sync.dma_start(
                o_out[tt * TT + u * C: tt * TT + (u + 1) * C, :], o_sb[:]
            )

        # software pipeline: tile tt's projections/gates interleave with
        # tile tt-1's chunk scan so the PE never idles long enough to be
        # HAM-throttled while the DVE works through the serial chain.
        prev = None
        for tt in range(ntt + 1):
            if tt < ntt:
                xt = xpool.tile([128, NKT, TT], BF16, tag="xt")
                nc.sync.dma_start(
                    xt[:],
                    xT[:, tt * TT:(tt + 1) * TT].rearrange(
                        "(k p) n -> p k n", p=128
                    ),
                )
                vtm = work.tile([C, NCHUNK, nh * DI], BF16, tag="vtm")
                hd = {}
                proj_items = (
                    [lambda b=b: emit_vblock(xt, vtm, b) for b in range(NBLK)]
                    + [lambda h=h, f=f: f(xt, hd, h)
                       for h in range(nh) for f in (emit_zq, emit_zf)]
                )
            else:
                proj_items = []

            st = {}
            for i, item in enumerate(proj_items):
                if prev is not None and i < NCHUNK:
                    hdp, vtmp, ttp = prev
                    if i >= 2:
                        emit_chunk_b(hdp, st, vtmp, ttp, i - 2)
                    emit_chunk_a(hdp, st, i)
                item()
            if prev is not None:
                hdp, vtmp, ttp = prev
                if proj_items:
                    emit_chunk_b(hdp, st, vtmp, ttp, NCHUNK - 2)
                    emit_chunk_b(hdp, st, vtmp, ttp, NCHUNK - 1)
                else:
                    # drain tile: no projections left to interleave
                    for u in range(NCHUNK):
                        emit_chunk_a(hdp, st, u)
                        emit_chunk_b(hdp, st, vtmp, ttp, u)
            prev = (hd, vtm, tt) if tt < ntt else None

    nc.compile()
    return nc


def _build_phase2():
    nc = _mk_nc()
    o_in = nc.dram_tensor("o2", [ROWS2, D], BF16, kind="ExternalInput")
    woT = nc.dram_tensor("woT", [D, D], BF16, kind="ExternalInput")
    ident = nc.dram_tensor("ident", [128, 128], BF16, kind="ExternalInput")
    y = nc.dram_tensor("y", [ROWS2, D], BF16, kind="ExternalOutput")

    with ExitStack() as ctx:
        tc = ctx.enter_context(tile.TileContext(nc))
        const = ctx.enter_context(tc.tile_pool(name="const", bufs=1))
        wpool = ctx.enter_context(tc.tile_pool(name="w", bufs=1))
        work = ctx.enter_context(tc.tile_pool(name="work", bufs=3))
        small = ctx.enter_context(tc.tile_pool(name="small", bufs=4))
        ps_tr = ctx.enter_context(tc.tile_pool(name="ps_tr", bufs=3, space=PSUM))
        ps_y = ctx.enter_context(tc.tile_pool(name="ps_y", bufs=3, space=PSUM))

        id_sb = const.tile([128, 128], BF16, tag="id")
        nc.sync.dma_start(id_sb[:], ident[:])
        eps_sb = const.tile([128, 1], F32, tag="eps")
        nc.vector.memset(eps_sb[:], EPS)
        wo_sb = wpool.tile([128, NKT, D], BF16, tag="wo")
        nc.sync.dma_start(wo_sb[:], woT[:].rearrange("(k p) m -> p k m", p=128))

        nblocks = ROWS2 // 128
        pend = None  # (onT tile, block index) awaiting matmuls
        for i in range(nblocks):
            ot = work.tile([128, D], BF16, tag="ot")
            nc.sync.dma_start(ot[:], o_in[i * 128:(i + 1) * 128, :])
            sq = work.tile([128, D], BF16, tag="sq")
            ssq = small.tile([128, 1], F32, tag="ssq")
            nc.scalar.activation(sq[:], ot[:], AF.Square, accum_out=ssq[:])
            nrm = small.tile([128, 1], F32, tag="nrm")
            nc.scalar.activation(nrm[:], ssq[:], AF.Sqrt, scale=1.0 / D, bias=eps_sb[:])
            inv = small.tile([128, 1], F32, tag="inv")
            nc.vector.reciprocal(inv[:], nrm[:])
            on = work.tile([128, D], BF16, tag="on")
            nc.vector.tensor_scalar(on[:], ot[:], inv[:], None, OP.mult)

            onT = work.tile([128, NKT, 128], BF16, tag="onT")
            for j in range(NKT):
                tp = ps_tr.tile([128, 128], BF16, tag="tr")
                nc.tensor.transpose(tp[:], on[:, j * 128:(j + 1) * 128], id_sb[:])
                nc.vector.tensor_copy(onT[:, j, :], tp[:])

            if pend is not None:
                _emit_oproj(nc, ps_y, work, wo_sb, y, *pend)
            pend = (onT, i)
        _emit_oproj(nc, ps_y, work, wo_sb, y, *pend)

    nc.compile()
    return nc


def _emit_oproj(nc, ps_y, work, wo_sb, y, onT, i):
    for n in range(D // 512):
        yp = ps_y.tile([128, 512], F32, tag="y", name="yp")
        for j in range(NKT):
            nc.tensor.matmul(
                yp[:], onT[:, j, :], wo_sb[:, j, n * 512:(n + 1) * 512],
                start=(j == 0), stop=(j == NKT - 1),
            )
        ysb = work.tile([128, 512], BF16, tag="ysb", name="ysb")
        nc.scalar.copy(ysb[:], yp[:])
        nc.sync.dma_start(
            y[i * 128:(i + 1) * 128, n * 512:(n + 1) * 512], ysb[:]
        )


_CACHE = {}
LAST_RESULTS = []
TRACE = False


def kernel(**inputs):
    x = np.asarray(inputs["hidden_states"], dtype=np.float32)
    Wq = np.asarray(inputs["Wq"], dtype=np.float32)
    Wf = np.asarray(inputs["Wf"], dtype=np.float32)
    Wi = np.asarray(inputs["Wi"], dtype=np.float32)
    gw = np.asarray(inputs["g_weight"], dtype=np.float32)
    Wo = np.asarray(inputs["Wo"], dtype=np.float32)

    if "p1" not in _CACHE:
        _CACHE["p1"] = _build_phase1()
    if "p2" not in _CACHE:
        _CACHE["p2"] = _build_phase2()

    ident = np.eye(128, dtype=NPBF16)
    tri = np.triu(np.ones((C, C), dtype=np.float32))
    maskT = np.concatenate([tri] * NH, axis=1)
    seg = np.tile(
        (np.arange(TT) % C != 0).astype(np.float32)[None, :], (128, 1)
    )

    xb = [np.ascontiguousarray(x[b].T.astype(NPBF16)) for b in range(B)]
    core_ids = list(range(NCORES))
    in_maps1 = []
    for c in core_ids:
        b, hp = c // 4, c % 4
        rs = slice(256 * hp, 256 * hp + 256)
        in_maps1.append({
            "xT": xb[b],
            "wqT": np.ascontiguousarray(Wq[rs].T.astype(NPBF16)),
            "wfT": np.ascontiguousarray(Wf[rs].T.astype(NPBF16)),
            "wiT": np.ascontiguousarray(Wi[rs].T.astype(NPBF16)),
            "ident": ident,
            "maskT": maskT,
            "seg": seg,
        })
    r1 = run_bass_kernel_spmd(_CACHE["p1"], in_maps1, core_ids, trace=TRACE)

    o_full = np.empty((B, T, D), dtype=NPBF16)
    for c in core_ids:
        b, hp = c // 4, c % 4
        o_full[b, :, 256 * hp: 256 * hp + 256] = r1.results[c]["o"]
    of = o_full.reshape(B * T, D)

    woT = np.ascontiguousarray((Wo * gw[None, :]).T.astype(NPBF16))
    in_maps2 = [
        {
            "o2": np.ascontiguousarray(of[c * ROWS2:(c + 1) * ROWS2]),
            "woT": woT,
            "ident": ident,
        }
        for c in core_ids
    ]
    r2 = run_bass_kernel_spmd(_CACHE["p2"], in_maps2, core_ids, trace=TRACE)

    LAST_RESULTS.clear()
    LAST_RESULTS.extend([r1, r2])

    out = np.concatenate([r2.results[c]["y"] for c in core_ids], axis=0)
    return out.reshape(B, T, D).astype(np.float32)


# revision 46
# speedup vs baseline: 1.2656x; 1.2656x over previous
"""HGRN2 attention forward on 8 Trainium2 NeuronCores.

Sharding: phase 1 is head-parallel (16 (batch, head) pairs -> 2 per core),
phase 2 is token-parallel (8192 token rows -> 1024 per core). The host
reshuffles the per-head scan outputs between the two SPMD launches.

The sequential gated scan is evaluated chunk-parallel (chunk C=64) with a
factored ("pre-scale") state P where the true state is S_u = egcC_u * P_u:
  P_{u+1} = egcC_u * P_u + kt_{u+1}^T v_{u+1}      (one STT op per head)
  o_u     = scale * (tril(qt^T kt) @ v + (qt * egcC_{u-1})^T @ P_{u-1})
with egc the per-chunk inclusive cumprod of sigmoid(z_f), qt = silu(z_q)*egc,
kt = (1-sigmoid(z_f))/egc. All matmuls run on the TensorEngine in bf16
(fp32 PSUM accumulation); v is produced token-major directly by using the
x tile as the stationary operand; the two heads share PSUM tiles for the
masked-attention, k-transpose and output stages so DVE/ACT ops and DMAs
are issued once per chunk instead of once per (chunk, head).
"""

import numpy as np
import ml_dtypes
from contextlib import ExitStack

import concourse.bass as bass
import concourse.mybir as mybir
import concourse.tile as tile
from concourse import bacc
from concourse.bass_utils import run_bass_kernel_spmd

F32 = mybir.dt.float32
BF16 = mybir.dt.bfloat16
AF = mybir.ActivationFunctionType
OP = mybir.AluOpType
PSUM = bass.MemorySpace.PSUM

B, T, D = 2, 4096, 1024
H, DF, DI = 8, 128, 128
EPS = 1e-5
SCALE = float(DF) ** -0.5
NCORES = 8
NH = 2              # heads per core
C = 64              # scan chunk length
TT = 512            # phase-1 token tile
NKT = D // 128      # contraction tiles
NTT = T // TT       # token tiles per core (phase 1)
NCHUNK = TT // C    # chunks per token tile
NBLK = TT // 128    # 128-token blocks per tile (token-major v)
ROWS2 = (B * T) // NCORES  # phase-2 token rows per core

NPBF16 = ml_dtypes.bfloat16


def _mk_nc():
    return bacc.Bacc(
        "TRN2",
        target_bir_lowering=False,
        debug=False,
        num_devices=NCORES,
    )


def _build_phase1(ntt=NTT, nh=NH):
    nc = _mk_nc()
    t_len = ntt * TT
    xT = nc.dram_tensor("xT", [D, t_len], BF16, kind="ExternalInput")
    wqT = nc.dram_tensor("wqT", [D, nh * DF], BF16, kind="ExternalInput")
    wfT = nc.dram_tensor("wfT", [D, nh * DF], BF16, kind="ExternalInput")
    wiT = nc.dram_tensor("wiT", [D, nh * DI], BF16, kind="ExternalInput")
    ident = nc.dram_tensor("ident", [128, 128], BF16, kind="ExternalInput")
    maskT = nc.dram_tensor("maskT", [C, nh * C], F32, kind="ExternalInput")
    seg = nc.dram_tensor("seg", [128, TT], F32, kind="ExternalInput")
    o_out = nc.dram_tensor("o", [t_len, nh * DI], BF16, kind="ExternalOutput")

    with ExitStack() as ctx:
        tc = ctx.enter_context(tile.TileContext(nc))
        const = ctx.enter_context(tc.tile_pool(name="const", bufs=1))
        wpool = ctx.enter_context(tc.tile_pool(name="w", bufs=1))
        xpool = ctx.enter_context(tc.tile_pool(name="x", bufs=2))
        work = ctx.enter_context(tc.tile_pool(name="work", bufs=3))
        small = ctx.enter_context(tc.tile_pool(name="small", bufs=6))
        spool = ctx.enter_context(tc.tile_pool(name="state", bufs=2))
        ps_proj = ctx.enter_context(tc.tile_pool(name="ps_proj", bufs=2, space=PSUM))
        ps_v = ctx.enter_context(tc.tile_pool(name="ps_v", bufs=1, space=PSUM))
        ps_o = ctx.enter_context(tc.tile_pool(name="ps_o", bufs=1, space=PSUM))
        ps_at = ctx.enter_context(tc.tile_pool(name="ps_at", bufs=1, space=PSUM))
        ps_tr = ctx.enter_context(tc.tile_pool(name="ps_tr", bufs=1, space=PSUM))
        ps_s = ctx.enter_context(tc.tile_pool(name="ps_s", bufs=2, space=PSUM))

        id_sb = const.tile([128, 128], BF16, tag="id")
        nc.sync.dma_start(id_sb[:], ident[:])
        mT_sb = const.tile([C, nh * C], F32, tag="mT")
        nc.sync.dma_start(mT_sb[:], maskT[:])
        seg_sb = const.tile([128, TT], F32, tag="seg")
        nc.sync.dma_start(seg_sb[:], seg[:])

        w_sb = {}
        for name, dram in (("q", wqT), ("f", wfT), ("i", wiT)):
            wt = wpool.tile([128, NKT, nh * DF], BF16, tag=f"w{name}")
            nc.sync.dma_start(wt[:], dram[:].rearrange("(k p) m -> p k m", p=128))
            w_sb[name] = wt

        P = []          # factored state per head (true S = egcC * P)
        eglast = []     # egc at last token of previous tile, per head
        for h in range(nh):
            s0 = spool.tile([DF, DI], BF16, tag=f"s{h}")
            nc.vector.memset(s0[:], 0.0)
            P.append(s0)
            el = spool.tile([128, 1], F32, tag=f"el{h}")
            nc.vector.memset(el[:], 1.0)
            eglast.append(el)

        def emit_vblock(xt, vtm, blk):
            vt_ps = ps_v.tile([128, nh * DI], F32, tag="v", name="vt_ps")
            for kt in range(NKT):
                nc.tensor.matmul(
                    vt_ps[:],
                    xt[:, kt, blk * 128:(blk + 1) * 128],
                    w_sb["i"][:, kt, :],
                    start=(kt == 0), stop=(kt == NKT - 1),
                )
            nc.scalar.copy(vtm[:, 2 * blk, :], vt_ps[0:C, :])
            nc.scalar.copy(vtm[:, 2 * blk + 1, :], vt_ps[C:128, :])

        def emit_zq(xt, hd, h):
            hs = slice(h * DF, (h + 1) * DF)
            zq = ps_proj.tile([128, TT], F32, tag="proj")
            for kt in range(NKT):
                nc.tensor.matmul(
                    zq[:], w_sb["q"][:, kt, hs], xt[:, kt, :],
                    start=(kt == 0), stop=(kt == NKT - 1),
                )
            # single ACT reader so the PSUM bank frees fast; Silu and Tanh
            # share one table set (silu_and_others) -> no table switches
            q_sb = work.tile([128, TT], F32, tag=f"q{h}")
            nc.scalar.activation(q_sb[:], zq[:], AF.Silu)
            hd[("q", h)] = q_sb

        def emit_zf(xt, hd, h):
            hs = slice(h * DF, (h + 1) * DF)
            zf = ps_proj.tile([128, TT], F32, tag="proj")
            for kt in range(NKT):
                nc.tensor.matmul(
                    zf[:], w_sb["f"][:, kt, hs], xt[:, kt, :],
                    start=(kt == 0), stop=(kt == NKT - 1),
                )
            # sigmoid via tanh (same ACT table as Silu):
            # sig = 0.5*tanh(z/2) + 0.5, k = 1 - sig = -0.5*tanh(z/2) + 0.5
            th = work.tile([128, TT], F32, tag="th")
            nc.scalar.activation(th[:], zf[:], AF.Tanh, scale=0.5)
            sig = work.tile([128, TT], F32, tag="sig")
            nc.vector.tensor_scalar(sig[:], th[:], 0.5, 0.5, OP.mult, OP.add)
            k_sb = work.tile([128, TT], F32, tag="k")
            nc.gpsimd.tensor_scalar(
                k_sb[:], th[:], -0.5, 0.5, OP.mult, OP.add
            )
            # per-chunk inclusive cumprod of sigmoid(zf) via scan with
            # reset at chunk starts (seg==0 there); prep on GpSimd
            d0 = work.tile([128, TT], F32, tag="d0")
            nc.gpsimd.tensor_tensor(d0[:], sig[:], seg_sb[:], OP.mult)
            d1 = work.tile([128, TT], F32, tag="d1")
            nc.gpsimd.tensor_tensor(d1[:], sig[:], d0[:], OP.subtract)
            egc = work.tile([128, TT], F32, tag=f"egc{h}")
            nc.vector.tensor_tensor_scan(
                egc[:], d0[:], d1[:], 0.0, OP.mult, OP.add
            )
            ep = work.tile([128, TT], F32, tag="ep")
            nc.vector.reciprocal_approx_fast(ep[:], egc[:])
            q_sb = hd[("q", h)]
            qt_sb = work.tile([128, TT], BF16, tag=f"qt{h}")
            nc.vector.tensor_tensor(qt_sb[:], q_sb[:], egc[:], OP.mult)
            kt_sb = work.tile([128, TT], BF16, tag=f"kt{h}")
            nc.vector.tensor_tensor(kt_sb[:], k_sb[:], ep[:], OP.mult)
            # qtp_t = qt_t * egcC_{chunk(t)-1}: extra decay from chunk entry,
            # for the o += qtp^T P term
            qtp_sb = work.tile([128, TT], BF16, tag=f"qtp{h}")
            nc.vector.tensor_scalar(
                qtp_sb[:, 0:C], qt_sb[:, 0:C], eglast[h][:], None, OP.mult
            )
            egr = egc[:].rearrange("p (c t) -> p c t", c=NCHUNK)
            egv = egr[:, 0:NCHUNK - 1, C - 1:C].broadcast_to(
                (128, NCHUNK - 1, C)
            )
            nc.vector.tensor_tensor(
                qtp_sb[:].rearrange("p (c t) -> p c t", c=NCHUNK)[:, 1:NCHUNK, :],
                qt_sb[:].rearrange("p (c t) -> p c t", c=NCHUNK)[:, 1:NCHUNK, :],
                egv, OP.mult,
            )
            hd[h] = (qt_sb, kt_sb, qtp_sb, egc, eglast[h])
            # next tile's gates and chunks use this tile's last egc column
            el = spool.tile([128, 1], F32, tag=f"el{h}")
            nc.gpsimd.tensor_copy(el[:], egc[:, TT - 1:TT])
            eglast[h] = el

        def emit_chunk_a(hd, st, u):
            # A^T[s,t] per head, side by side in one PSUM tile
            at_ps = ps_at.tile([C, nh * C], F32, tag="at")
            sl = slice(u * C, (u + 1) * C)
            for h in range(nh):
                nc.tensor.matmul(
                    at_ps[:, h * C:(h + 1) * C],
                    hd[h][1][:, sl], hd[h][0][:, sl],
                    start=True, stop=True,
                )
            atm = small.tile([C, nh * C], BF16, tag="atm")
            nc.vector.tensor_tensor(atm[:], at_ps[:], mT_sb[:], OP.mult)

            # token-major k for both heads via PE transpose
            kht_ps = ps_tr.tile([C, nh * DF], BF16, tag="tr")
            for h in range(nh):
                nc.tensor.transpose(
                    kht_ps[:, h * DF:(h + 1) * DF], hd[h][1][:, sl], id_sb[:]
                )
            kht = small.tile([C, nh * DF], BF16, tag="kht")
            nc.scalar.copy(kht[:], kht_ps[:])
            st[u] = (atm, kht)

        def emit_chunk_b(hd, st, vtm, tt, u):
            atm, kht = st.pop(u)
            sl = slice(u * C, (u + 1) * C)
            # o = A @ v + qtp^T @ P (token-major [C, nh*DI])
            o_ps = ps_o.tile([C, nh * DI], F32, tag="o", name="o_ps")
            for h in range(nh):
                od = slice(h * DI, (h + 1) * DI)
                nc.tensor.matmul(
                    o_ps[:, od], atm[:, h * C:(h + 1) * C], vtm[:, u, od],
                    start=True, stop=False,
                )
                nc.tensor.matmul(
                    o_ps[:, od], hd[h][2][:, sl], P[h][:],
                    start=False, stop=True,
                )

            # P' = egcC_{u-1} * P + kt^T @ v
            for h in range(nh):
                od = slice(h * DI, (h + 1) * DI)
                s_ps = ps_s.tile([DF, DI], F32, tag="sps")
                nc.tensor.matmul(
                    s_ps[:], kht[:, h * DF:(h + 1) * DF], vtm[:, u, od],
                    start=True, stop=True,
                )
                egc = hd[h][3]
                dec = hd[h][4][:] if u == 0 else egc[:, u * C - 1:u * C]
                s_new = spool.tile([DF, DI], BF16, tag=f"s{h}")
                nc.vector.scalar_tensor_tensor(
                    s_new[:], P[h][:], dec, s_ps[:], OP.mult, OP.add,
                )
                P[h] = s_new

            o_sb = small.tile([C, nh * DI], BF16, tag="osb")
            nc.scalar.activation(o_sb[:], o_ps[:], AF.Copy, scale=SCALE)
            nc.sync.dma_start(
                o_out[tt * TT + u * C: tt * TT + (u + 1) * C, :], o_sb[:]
            )

        # software pipeline: tile tt's projections/gates interleave with
        # tile tt-1's chunk scan so the PE never idles long enough to be
        # HAM-throttled while the DVE works through the serial chain.
        prev = None
        for tt in range(ntt + 1):
            if tt < ntt:
                xt = xpool.tile([128, NKT, TT], BF16, tag="xt")
                nc.sync.dma_start(
                    xt[:],
                    xT[:, tt * TT:(tt + 1) * TT].rearrange(
                        "(k p) n -> p k n", p=128
                    ),
                )
                vtm = work.tile([C, NCHUNK, nh * DI], BF16, tag="vtm")
                hd = {}
                proj_items = (
                    [lambda b=b: emit_vblock(xt, vtm, b) for b in range(NBLK)]
                    + [lambda h=h, f=f: f(xt, hd, h)
                       for h in range(nh) for f in (emit_zq, emit_zf)]
                )
            else:
                proj_items = []

            st = {}
            for i, item in enumerate(proj_items):
                if prev is not None and i < NCHUNK:
                    hdp, vtmp, ttp = prev
                    if i >= 2:
                        emit_chunk_b(hdp, st, vtmp, ttp, i - 2)
                    emit_chunk_a(hdp, st, i)
                item()
            if prev is not None:
                hdp, vtmp, ttp = prev
                if proj_items:
                    emit_chunk_b(hdp, st, vtmp, ttp, NCHUNK - 2)
                    emit_chunk_b(hdp, st, vtmp, ttp, NCHUNK - 1)
                else:
                    # drain tile: no projections left to interleave
                    for u in range(NCHUNK):
                        emit_chunk_a(hdp, st, u)
                        emit_chunk_b(hdp, st, vtmp, ttp, u)
            prev = (hd, vtm, tt) if tt < ntt else None

    nc.compile()
    return nc


def _build_phase2():
    nc = _mk_nc()
    o_in = nc.dram_tensor("o2", [ROWS2, D], BF16, kind="ExternalInput")
    woT = nc.dram_tensor("woT", [D, D], BF16, kind="ExternalInput")
    ident = nc.dram_tensor("ident", [128, 128], BF16, kind="ExternalInput")
    y = nc.dram_tensor("y", [ROWS2, D], BF16, kind="ExternalOutput")

    with ExitStack() as ctx:
        tc = ctx.enter_context(tile.TileContext(nc))
        const = ctx.enter_context(tc.tile_pool(name="const", bufs=1))
        wpool = ctx.enter_context(tc.tile_pool(name="w", bufs=1))
        work = ctx.enter_context(tc.tile_pool(name="work", bufs=3))
        small = ctx.enter_context(tc.tile_pool(name="small", bufs=4))
        ps_tr = ctx.enter_context(tc.tile_pool(name="ps_tr", bufs=3, space=PSUM))
        ps_y = ctx.enter_context(tc.tile_pool(name="ps_y", bufs=3, space=PSUM))

        id_sb = const.tile([128, 128], BF16, tag="id")
        nc.sync.dma_start(id_sb[:], ident[:])
        eps_sb = const.tile([128, 1], F32, tag="eps")
        nc.vector.memset(eps_sb[:], EPS)
        wo_sb = wpool.tile([128, NKT, D], BF16, tag="wo")
        nc.sync.dma_start(wo_sb[:], woT[:].rearrange("(k p) m -> p k m", p=128))

        nblocks = ROWS2 // 128
        pend = None  # (onT tile, block index) awaiting matmuls
        for i in range(nblocks):
            ot = work.tile([128, D], BF16, tag="ot")
            nc.sync.dma_start(ot[:], o_in[i * 128:(i + 1) * 128, :])
            sq = work.tile([128, D], BF16, tag="sq")
            ssq = small.tile([128, 1], F32, tag="ssq")
            nc.scalar.activation(sq[:], ot[:], AF.Square, accum_out=ssq[:])
            nrm = small.tile([128, 1], F32, tag="nrm")
            nc.scalar.activation(nrm[:], ssq[:], AF.Sqrt, scale=1.0 / D, bias=eps_sb[:])
            inv = small.tile([128, 1], F32, tag="inv")
            nc.vector.reciprocal(inv[:], nrm[:])
            on = work.tile([128, D], BF16, tag="on")
            nc.vector.tensor_scalar(on[:], ot[:], inv[:], None, OP.mult)

            onT = work.tile([128, NKT, 128], BF16, tag="onT")
            for j in range(NKT):
                tp = ps_tr.tile([128, 128], BF16, tag="tr")
                nc.tensor.transpose(tp[:], on[:, j * 128:(j + 1) * 128], id_sb[:])
                nc.vector.tensor_copy(onT[:, j, :], tp[:])

            if pend is not None:
                _emit_oproj(nc, ps_y, work, wo_sb, y, *pend)
            pend = (onT, i)
        _emit_oproj(nc, ps_y, work, wo_sb, y, *pend)

    nc.compile()
    return nc


def _emit_oproj(nc, ps_y, work, wo_sb, y, onT, i):
    for n in range(D // 512):
        yp = ps_y.tile([128, 512], F32, tag="y", name="yp")
        for j in range(NKT):
            nc.tensor.matmul(
                yp[:], onT[:, j, :], wo_sb[:, j, n * 512:(n + 1) * 512],
                start=(j == 0), stop=(j == NKT - 1),
            )
        ysb = work.tile([128, 512], BF16, tag="ysb", name="ysb")
        nc.scalar.copy(ysb[:], yp[:])
        nc.sync.dma_start(
            y[i * 128:(i + 1) * 128, n * 512:(n + 1) * 512], ysb[:]
        )


_CACHE = {}
LAST_RESULTS = []
TRACE = False


def kernel(**inputs):
    x = np.asarray(inputs["hidden_states"], dtype=np.float32)
    Wq = np.asarray(inputs["Wq"], dtype=np.float32)
    Wf = np.asarray(inputs["Wf"], dtype=np.float32)
    Wi = np.asarray(inputs["Wi"], dtype=np.float32)
    gw = np.asarray(inputs["g_weight"], dtype=np.float32)
    Wo = np.asarray(inputs["Wo"], dtype=np.float32)

    if "p1" not in _CACHE:
        _CACHE["p1"] = _build_phase1()
    if "p2" not in _CACHE:
        _CACHE["p2"] = _build_phase2()

    ident = np.eye(128, dtype=NPBF16)
    tri = np.triu(np.ones((C, C), dtype=np.float32))
    maskT = np.concatenate([tri] * NH, axis=1)
    seg = np.tile(
        (np.arange(TT) % C != 0).astype(np.float32)[None, :], (128, 1)
    )

    xb = [np.ascontiguousarray(x[b].T.astype(NPBF16)) for b in range(B)]
    core_ids = list(range(NCORES))
    in_maps1 = []
    for c in core_ids:
        b, hp = c // 4, c % 4
        rs = slice(256 * hp, 256 * hp + 256)
        in_maps1.append({
            "xT": xb[b],
            "wqT": np.ascontiguousarray(Wq[rs].T.astype(NPBF16)),
            "wfT": np.ascontiguousarray(Wf[rs].T.astype(NPBF16)),
            "wiT": np.ascontiguousarray(Wi[rs].T.astype(NPBF16)),
            "ident": ident,
            "maskT": maskT,
            "seg": seg,
        })
    r1 = run_bass_kernel_spmd(_CACHE["p1"], in_maps1, core_ids, trace=TRACE)

    o_full = np.empty((B, T, D), dtype=NPBF16)
    for c in core_ids:
        b, hp = c // 4, c % 4
        o_full[b, :, 256 * hp: 256 * hp + 256] = r1.results[c]["o"]
    of = o_full.reshape(B * T, D)

    woT = np.ascontiguousarray((Wo * gw[None, :]).T.astype(NPBF16))
    in_maps2 = [
        {
            "o2": np.ascontiguousarray(of[c * ROWS2:(c + 1) * ROWS2]),
            "woT": woT,
            "ident": ident,
        }
        for c in core_ids
    ]
    r2 = run_bass_kernel_spmd(_CACHE["p2"], in_maps2, core_ids, trace=TRACE)

    LAST_RESULTS.clear()
    LAST_RESULTS.extend([r1, r2])

    out = np.concatenate([r2.results[c]["y"] for c in core_ids], axis=0)
    return out.reshape(B, T, D).astype(np.float32)
